# revision 1
# baseline (speedup 1.0000x reference)
"""Trainium2 Bass kernel for GCNGraphClassifier (3x GCNConv+GraphNorm+ReLU,
global_mean_pool, linear head). 8-core SPMD.

Self-contained: host preprocessing (graph partitioning, banded edge
schedule, norm factorization) + Bass/Tile device program.

Sharding: graphs block-partitioned to cores (32/core); each graph padded
to a 512-column window; node columns degree-packed into 16 col-blocks with
a globally uniform tile allotment so the SPMD instruction stream is
identical across cores. Message aggregation = PE matmuls against a fixed
0/1 pattern; gathers via indirect DMA from a replicated bf16 node table
(AllGather between layers). GCN norm factorized into per-node pre/post
scales (dinv), so edge messages need no per-edge multiply.
"""

import os
import numpy as np

N = 100000
ERAW = 1600000
G = 256
NCORES = 8
GPC = 32
WCOLS = 512
CPC = GPC * WCOLS  # 16384
EPS = 1e-5
FINS = (4, 32, 64)
FOUTS = (32, 64, 128)


# ------------------------------------------------------------------ host prep
def _prep(edge_index, batch):
    src = np.asarray(edge_index[0], dtype=np.int64)
    dst = np.asarray(edge_index[1], dtype=np.int64)
    batch = np.asarray(batch, dtype=np.int64)
    loops = np.arange(N, dtype=np.int64)
    src = np.concatenate([src, loops])
    dst = np.concatenate([dst, loops])
    deg = np.bincount(dst, minlength=N)
    dinv = (1.0 / np.sqrt(np.maximum(deg, 1.0))).astype(np.float64)
    dinv[deg == 0] = 0.0

    order = np.argsort(dst, kind="stable")
    src_s = src[order]
    dst_s = dst[order]
    starts = np.zeros(N + 1, dtype=np.int64)
    np.cumsum(np.bincount(dst, minlength=N), out=starts[1:])

    gcnt = np.bincount(batch, minlength=G).astype(np.int64)
    gstart = np.zeros(G + 1, dtype=np.int64)
    np.cumsum(gcnt, out=gstart[1:])
    assert gcnt.max() <= WCOLS

    need = -(-deg // 4)  # ceil(deg/4), tiles per node

    # column packing: per graph, nodes sorted by degree desc -> col ranks
    col_of = np.full(N, -1, dtype=np.int64)
    node_at = np.full((G, WCOLS), -1, dtype=np.int64)
    blockneed = np.zeros((G, 16), dtype=np.int64)
    for g in range(G):
        nodes = np.arange(gstart[g], gstart[g + 1])
        nodes = nodes[np.argsort(-deg[nodes], kind="stable")]
        cols = np.arange(len(nodes))
        col_of[nodes] = cols
        node_at[g, : len(nodes)] = nodes
        np.maximum.at(blockneed[g], cols // 32, need[nodes])
    A = blockneed.max(axis=0)
    T_win = int(A.sum())
    tstart = np.zeros(17, dtype=np.int64)
    np.cumsum(A, out=tstart[1:])
    tile_block = np.repeat(np.arange(16), A)
    T_total = T_win * GPC

    core_of = batch // GPC
    row_of = core_of * CPC + (batch % GPC) * WCOLS + col_of
    PAD_ROW = WCOLS - 1  # core0 win0 col511: guaranteed pad (dinv=0 -> zeros)
    assert node_at[0, WCOLS - 1] == -1

    # vectorized per-edge slot assignment
    e_node = dst_s
    e_rank = np.arange(len(dst_s)) - starts[e_node]
    e_core = core_of[e_node]
    e_w = (batch[e_node] % GPC)
    e_col = col_of[e_node]
    e_b = e_col // 32
    e_ci = e_col % 32
    e_k = e_rank // 4
    e_j = e_rank % 4
    e_t = e_w * T_win + tstart[e_b] + e_k
    e_r = 4 * e_ci + e_j
    offs = np.full((NCORES, 128, T_total), PAD_ROW, dtype=np.int32)
    offs[e_core, e_r, e_t] = row_of[src_s].astype(np.int32)
    return dict(dinv=dinv, gcnt=gcnt, node_at=node_at, A=A, T_win=T_win,
                T_total=T_total, tile_block=tile_block, offs=offs,
                e=(e_core, e_r, e_t, src_s), batch=batch)


def _core_inputs(S, x):
    import ml_dtypes
    dinv = S["dinv"]
    T_total = S["T_total"]
    x4 = np.zeros((N, 4), np.float32)
    x4[:, :3] = x
    gx = (dinv[:, None] * x4).astype(np.float32)
    e_core, e_r, e_t, src_s = S["e"]

    xe_all = np.zeros((NCORES, 128, T_total, 4), np.float32)
    xe_all[e_core, e_r, e_t] = gx[src_s]

    cores = []
    for c in range(NCORES):
        dcols = np.zeros(CPC, np.float64)
        for w in range(GPC):
            nodes = S["node_at"][c * GPC + w]
            msk = nodes >= 0
            dcols[w * WCOLS:(w + 1) * WCOLS][msk] = dinv[nodes[msk]]
        gcnt_c = S["gcnt"][c * GPC:(c + 1) * GPC].astype(np.float32)
        invcnt = (1.0 / np.maximum(gcnt_c, 1.0)).astype(np.float32)
        npad = (WCOLS - gcnt_c).astype(np.float32)
        cores.append(dict(
            xe=np.ascontiguousarray(xe_all[c].reshape(128, T_total * 4)).astype(ml_dtypes.bfloat16),
            offs=np.ascontiguousarray(S["offs"][c]),
            dinvb=np.broadcast_to(dcols.astype(np.float32), (64, CPC)).copy(),
            dinv_nm=np.ascontiguousarray(
                dcols.astype(np.float32).reshape(CPC // 128, 128).T),
            invcnt_pb=np.broadcast_to(invcnt, (128, GPC)).copy(),
            npad_pb=np.broadcast_to(npad, (128, GPC)).copy(),
        ))
    return cores


def _split_multiwaits(nc):
    """This walrus build accepts at most one sync-wait per instruction
    struct; split extras onto same-engine NoOps inserted just before."""
    import concourse.mybir as mybir
    k = 0
    for bass_bb in nc.bb_map.values():
        bb = bass_bb.bb if hasattr(bass_bb, "bb") else bass_bb
        out = []
        changed = False
        for ins in bb.instructions:
            si = getattr(ins, "sync_info", None)
            if si is not None and si.on_wait is not None and len(si.on_wait) > 1:
                waits = list(si.on_wait)
                for wsub in waits[:-1]:
                    k += 1
                    nop = mybir.InstNoOp(name=f"WNOP-{k}", engine=ins.engine,
                                         ins=[], outs=[])
                    nop.sync_info = mybir.SyncInfo(on_wait=[wsub], on_update=[])
                    out.append(nop)
                ins.sync_info = mybir.SyncInfo(
                    on_wait=[waits[-1]], on_update=list(si.on_update))
                changed = True
            out.append(ins)
        if changed:
            bb.instructions = out


# --------------------------------------------------------------- bass program
def _build_nc(T_win, tile_block):
    import concourse.bass as bass
    import concourse.mybir as mybir
    from concourse.tile import TileContext
    from concourse.masks import make_identity

    f32 = mybir.dt.float32
    bf16 = mybir.dt.bfloat16
    i32 = mybir.dt.int32
    AX = mybir.AxisListType.X
    OP = mybir.AluOpType
    AF = mybir.ActivationFunctionType
    T_total = T_win * GPC

    nc = bass.Bass()
    xe_d = nc.declare_dram_parameter("xe", [128, T_total * 4], bf16, isOutput=False)
    offs_d = nc.declare_dram_parameter("offs", [128, T_total], i32, isOutput=False)
    dinvb_d = nc.declare_dram_parameter("dinvb", [64, CPC], f32, isOutput=False)
    dinvnm_d = nc.declare_dram_parameter("dinv_nm", [128, CPC // 128], f32, isOutput=False)
    invcnt_d = nc.declare_dram_parameter("invcnt_pb", [128, GPC], f32, isOutput=False)
    npad_d = nc.declare_dram_parameter("npad_pb", [128, GPC], f32, isOutput=False)
    w1_d = nc.declare_dram_parameter("w1", [4, 32], bf16, isOutput=False)
    w2_d = nc.declare_dram_parameter("w2", [32, 64], bf16, isOutput=False)
    w3_d = nc.declare_dram_parameter("w3", [64, 128], bf16, isOutput=False)
    linw_d = nc.declare_dram_parameter("linw", [128, 3], f32, isOutput=False)
    linb_d = nc.declare_dram_parameter("linb_pb", [32, 3], f32, isOutput=False)
    # gn consts per layer: cols = [neg_alpha, c=(1-a)b, w_gn, b_gn]
    gnc_d = [nc.declare_dram_parameter(f"gnc{i+1}", [FOUTS[i], 4], f32,
                                       isOutput=False) for i in range(3)]
    ofixf_d = nc.declare_dram_parameter("ofix_f", [128, 32], f32, isOutput=False)
    ofixh_d = nc.declare_dram_parameter("ofix_h", [128, 32], bf16, isOutput=False)
    out_d = nc.declare_dram_parameter("out", [32, 3], f32, isOutput=True)

    tbl_sh = [nc.dram_tensor(f"tbl{i}_sh", [CPC, FINS[i]], bf16)
              for i in (1, 2)]
    tbl_full = [nc.dram_tensor(f"tbl{i}_full", [NCORES * CPC, FINS[i]], bf16,
                               addr_space="Shared") for i in (1, 2)]

    RG = [list(range(NCORES))]

    with TileContext(nc) as tc:
        with (
            tc.tile_pool(name="const", bufs=1) as cpool,
            tc.tile_pool(name="big", bufs=1) as bigpool,
            tc.tile_pool(name="sb", bufs=2) as sb,
            tc.tile_pool(name="stat", bufs=12) as st,
            tc.tile_pool(name="ps_agg", bufs=2, space="PSUM") as ps_agg,
            tc.tile_pool(name="ps_dn", bufs=2, space="PSUM") as ps_dn,
            tc.tile_pool(name="ps_tp", bufs=2, space="PSUM") as ps_tp,
            tc.tile_pool(name="gpool", bufs=2) as gpool,
        ):
            # ---- constants to SBUF
            def load(pool, dram, shape, dtype, tag):
                t = pool.tile(shape, dtype, tag=tag)
                nc.sync.dma_start(out=t[:], in_=dram[:])
                return t

            ofix_f = load(cpool, ofixf_d, [128, 32], f32, "ofix_f")
            ofix_h = load(cpool, ofixh_d, [128, 32], bf16, "ofix_h")
            offs_sb = load(cpool, offs_d, [128, T_total], i32, "offs")
            dinvb = load(cpool, dinvb_d, [64, CPC], f32, "dinvb")
            dinvnm = load(cpool, dinvnm_d, [128, CPC // 128], f32, "dinvnm")
            invcnt = load(cpool, invcnt_d, [128, GPC], f32, "invcnt")
            npad = load(cpool, npad_d, [128, GPC], f32, "npad")
            w1 = load(cpool, w1_d, [4, 32], bf16, "w1")
            w2 = load(cpool, w2_d, [32, 64], bf16, "w2")
            w3 = load(cpool, w3_d, [64, 128], bf16, "w3")
            linw = load(cpool, linw_d, [128, 3], f32, "linw")
            linb = load(cpool, linb_d, [32, 3], f32, "linb")
            gnc = [load(cpool, gnc_d[i], [FOUTS[i], 4], f32, f"gnc{i}")
                   for i in range(3)]
            Ws = [w1, w2, w3]

            ident = cpool.tile([128, 128], f32)
            make_identity(nc, ident[:])
            zl = cpool.tile([128, 64], f32)
            nc.vector.memset(zl[:], 0.0)
            zn = cpool.tile([128, 512], f32)
            nc.vector.memset(zn[:], 0.0)
            zlh = cpool.tile([128, 64], bf16)
            nc.vector.memset(zlh[:], 0.0)
            znh = cpool.tile([128, 512], bf16)
            nc.vector.memset(znh[:], 0.0)
            epsc = cpool.tile([128, 1], f32)
            nc.vector.memset(epsc[:], EPS)

            # per-engine const warmups: absorb each const tile's DMA wait
            # onto its consuming engine once (walrus: <=1 sync wait per inst)
            scrd = cpool.tile([128, 8], f32, tag="scrd")
            for ap in (dinvb[:64, :1], dinvnm[:, :1], invcnt[:, :1],
                       npad[:, :1], linb[:32, :1], gnc[0][:, :1],
                       gnc[1][:, :1], gnc[2][:, :1], epsc[:, :1]):
                nc.vector.tensor_copy(out=scrd[:ap.shape[0], :1], in_=ap)
            scra = cpool.tile([128, 8], f32, tag="scra")
            nc.scalar.activation(out=scra[:, :1], in_=epsc[:, :], func=AF.Copy)
            for i in range(3):
                nc.scalar.activation(out=scra[:FOUTS[i], 1:2], in_=gnc[i][:, :1],
                                     func=AF.Copy)
            scrg = cpool.tile([128, 8], i32, tag="scrg")
            nc.gpsimd.tensor_copy(out=scrg[:, :1], in_=offs_sb[:, :1])

            # wait-absorbers: pull const-DMA/identity deps onto PE early so
            # no later matmul carries more than one sync wait (LW struct limit)
            tr0 = ps_tp.tile([128, 512], f32, tag="tp")
            nc.tensor.transpose(out=tr0[:, :128], in_=ident[:, :], identity=ident[:, :])
            nc.tensor.matmul(out=tr0[:3, :3], lhsT=linw[:, :], rhs=linw[:, :],
                             start=True, stop=True)
            nc.tensor.ldweights(weights=w1[:, :])
            nc.tensor.ldweights(weights=w2[:, :])
            nc.tensor.ldweights(weights=w3[:, :])
            nc.tensor.ldweights(weights=ofix_h[:, :])

            p_sb = bigpool.tile([64, CPC], bf16)      # aggregation out (post-scaled)
            pooled = bigpool.tile([128, GPC], f32)   # layer3 pooled sums

            # ---------------- helpers ----------------
            GRP = 4  # windows per gather group (SBUF: GRP*T_win*64*2B/part)

            def aggregate(lay, Fin):
                """phase 1: all gathers for the layer into one big buffer;
                phase 2: O-pattern matmuls per window; p_sb <- psum*dinv."""
                fp32 = lay == 0
                of = ofix_h
                zlx, znx = (zlh, znh)
                for g0 in range(0, GPC, GRP):
                    gbig = gpool.tile([128, GRP * T_win * 64], bf16, tag="gbig")
                    tbase = g0 * T_win
                    ngrp = GRP * T_win
                    if fp32:
                        nc.sync.dma_start(
                            out=gbig[:, :ngrp * 4],
                            in_=xe_d[:, tbase * 4:(tbase + ngrp) * 4])
                    elif os.environ.get("K_NOGATHER") == "1":
                        nc.sync.dma_start(
                            out=gbig[:, :ngrp * Fin],
                            in_=tbl_full[lay - 1][:ngrp, :]
                            .rearrange("t f -> () (t f)").to_broadcast(
                                [128, ngrp * Fin]))
                    else:
                        for t in range(ngrp):
                            nc.gpsimd.indirect_dma_start(
                                out=gbig[:, t * Fin:(t + 1) * Fin],
                                out_offset=None,
                                in_=tbl_full[lay - 1][:, :],
                                in_offset=bass.IndirectOffsetOnAxis(
                                    ap=offs_sb[:, tbase + t:tbase + t + 1],
                                    axis=0),
                            )
                    nc.tensor.ldweights(weights=gbig[:, :Fin])
                    for w in range(g0, g0 + GRP):
                        ps = ps_agg.tile([64, 512], f32, tag="agg")
                        nc.tensor.matmul(out=ps[:Fin, :], lhsT=zlx[:, :Fin],
                                         rhs=znx[:], start=True, stop=False)
                        for t in range(T_win):
                            b = tile_block[t]
                            tl = (w - g0) * T_win + t
                            nc.tensor.matmul(
                                out=ps[:Fin, 32 * b:32 * b + 32],
                                lhsT=gbig[:, tl * Fin:(tl + 1) * Fin],
                                rhs=of[:], start=False, stop=(t == T_win - 1))
                        nc.vector.tensor_tensor(
                            out=p_sb[:Fin, w * WCOLS:(w + 1) * WCOLS],
                            in0=ps[:Fin, :],
                            in1=dinvb[:Fin, w * WCOLS:(w + 1) * WCOLS],
                            op=OP.mult)

            def dense_gn(lay, Fin, Fo, last):
                W, gc = Ws[lay], gnc[lay]
                for w in range(GPC):
                    wsl = slice(w * WCOLS, (w + 1) * WCOLS)
                    px = ps_dn.tile([128, 512], f32, tag="dense")
                    nc.tensor.matmul(out=px[:Fo, :], lhsT=W[:, :],
                                     rhs=p_sb[:Fin, wsl], start=True, stop=True)
                    # stats via ACT accumulate
                    scr = sb.tile([128, 512], f32, tag="scr")
                    ssum = st.tile([128, 1], f32, tag="ssum")
                    ssq = st.tile([128, 1], f32, tag="ssq")
                    nc.scalar.activation(out=scr[:Fo, :], in_=px[:Fo, :],
                                         func=AF.Copy, accum_out=ssum[:Fo, :])
                    nc.scalar.activation(out=scr[:Fo, :], in_=px[:Fo, :],
                                         func=AF.Square, accum_out=ssq[:Fo, :])
                    # scalar math [Fo,1]
                    m = st.tile([128, 1], f32, tag="m")
                    qm = st.tile([128, 1], f32, tag="qm")
                    d = st.tile([128, 1], f32, tag="d")
                    t1 = st.tile([128, 1], f32, tag="t1")
                    var = st.tile([128, 1], f32, tag="var")
                    istd = st.tile([128, 1], f32, tag="istd")
                    s1 = st.tile([128, 1], f32, tag="s1")
                    s2 = st.tile([128, 1], f32, tag="s2")
                    nc.vector.tensor_scalar(out=m[:Fo], in0=ssum[:Fo],
                                            scalar1=invcnt[:Fo, w:w + 1],
                                            scalar2=None, op0=OP.mult)
                    nc.vector.tensor_scalar(out=qm[:Fo], in0=ssq[:Fo],
                                            scalar1=invcnt[:Fo, w:w + 1],
                                            scalar2=None, op0=OP.mult)
                    # d = m*neg_alpha + c
                    nc.vector.tensor_scalar(out=d[:Fo], in0=m[:Fo],
                                            scalar1=gc[:, 0:1], scalar2=gc[:, 1:2],
                                            op0=OP.mult, op1=OP.add)
                    # var = qm + d*(2m + d)
                    nc.vector.tensor_scalar(out=t1[:Fo], in0=m[:Fo], scalar1=2.0,
                                            scalar2=d[:Fo], op0=OP.mult, op1=OP.add)
                    nc.vector.tensor_tensor(out=t1[:Fo], in0=t1[:Fo], in1=d[:Fo],
                                            op=OP.mult)
                    nc.vector.tensor_tensor(out=var[:Fo], in0=qm[:Fo], in1=t1[:Fo],
                                            op=OP.add)
                    nc.scalar.activation(out=istd[:Fo], in_=var[:Fo], func=AF.Sqrt,
                                         bias=epsc[:Fo, :])
                    nc.vector.reciprocal(out=istd[:Fo], in_=istd[:Fo])
                    nc.vector.tensor_scalar(out=s1[:Fo], in0=istd[:Fo],
                                            scalar1=gc[:, 2:3], scalar2=None,
                                            op0=OP.mult)
                    nc.vector.tensor_scalar(out=s2[:Fo], in0=d[:Fo],
                                            scalar1=s1[:Fo], scalar2=gc[:, 3:4],
                                            op0=OP.mult, op1=OP.add)
                    # h_pre = px*s1 + s2 ; relu
                    hw = sb.tile([128, 512], f32, tag="hw")
                    nc.vector.tensor_scalar(out=hw[:Fo, :], in0=px[:Fo, :],
                                            scalar1=s1[:Fo], scalar2=s2[:Fo],
                                            op0=OP.mult, op1=OP.add)
                    if last:
                        hr = sb.tile([128, 512], f32, tag="hr")
                        psum_col = st.tile([128, 1], f32, tag="pool1")
                        nc.scalar.activation(out=hr[:Fo, :], in_=hw[:Fo, :],
                                             func=AF.Relu,
                                             accum_out=psum_col[:Fo, :])
                        # pooled -= npad * relu(s2); then *invcnt
                        rs2 = st.tile([128, 1], f32, tag="rs2")
                        nc.vector.tensor_scalar(out=rs2[:Fo], in0=s2[:Fo],
                                                scalar1=0.0, scalar2=npad[:Fo, w:w + 1],
                                                op0=OP.max, op1=OP.mult)
                        nc.vector.tensor_tensor(out=psum_col[:Fo], in0=psum_col[:Fo],
                                                in1=rs2[:Fo], op=OP.subtract)
                        nc.vector.tensor_scalar(out=pooled[:Fo, w:w + 1],
                                                in0=psum_col[:Fo],
                                                scalar1=invcnt[:Fo, w:w + 1],
                                                scalar2=None, op0=OP.mult)
                    else:
                        hr = sb.tile([128, 512], f32, tag="hr")
                        nc.scalar.activation(out=hr[:Fo, :], in_=hw[:Fo, :],
                                             func=AF.Relu)
                        # transpose 4x [Fo,128] -> [128,Fo], prescale, -> table
                        tp = ps_tp.tile([128, 512], f32, tag="tp")
                        tb = sb.tile([128, 4 * Fo], bf16, tag="tb")
                        for ccc in range(4):
                            nc.tensor.transpose(
                                out=tp[:, ccc * Fo:(ccc + 1) * Fo],
                                in_=hr[:Fo, 128 * ccc:128 * (ccc + 1)],
                                identity=ident[:Fo, :Fo])
                            nc.vector.tensor_scalar(
                                out=tb[:, ccc * Fo:(ccc + 1) * Fo],
                                in0=tp[:, ccc * Fo:(ccc + 1) * Fo],
                                scalar1=dinvnm[:, 4 * w + ccc:4 * w + ccc + 1],
                                scalar2=None, op0=OP.mult)
                        nc.sync.dma_start(
                            out=tbl_sh[lay][w * WCOLS:(w + 1) * WCOLS, :]
                            .rearrange("(c p) f -> p c f", p=128),
                            in_=tb[:].rearrange("p (c f) -> p c f", f=Fo))

            NOCC = os.environ.get("K_NOCC") == "1"
            NOGATHER = os.environ.get("K_NOGATHER") == "1"
            ONELAYER = os.environ.get("K_ONELAYER") == "1"

            def distribute(i):
                if NOCC:
                    # local-only stand-in (wrong answer, bisect aid)
                    nc.sync.dma_start(out=tbl_full[i][:CPC, :],
                                      in_=tbl_sh[i][:, :])
                else:
                    nc.gpsimd.collective_compute(
                        "AllGather", OP.bypass, ins=[tbl_sh[i][:, :]],
                        outs=[tbl_full[i][:, :]], replica_groups=RG)

            # ---------------- layer 1 ----------------
            aggregate(0, 4)
            dense_gn(0, 4, 32, last=False)
            if not ONELAYER:
                distribute(0)
                # ---------------- layer 2 ----------------
                aggregate(1, 32)
                dense_gn(1, 32, 64, last=False)
                distribute(1)
                # ---------------- layer 3 ----------------
                aggregate(2, 64)
                dense_gn(2, 64, 128, last=True)

            # ---------------- head ----------------
            hps = ps_tp.tile([32, 3], f32, tag="head")
            nc.tensor.matmul(out=hps[:, :], lhsT=pooled[:, :], rhs=linw[:, :],
                             start=True, stop=True)
            ob = sb.tile([32, 3], f32, tag="ob")
            nc.vector.tensor_tensor(out=ob[:], in0=hps[:], in1=linb[:], op=OP.add)
            nc.sync.dma_start(out=out_d[:, :], in_=ob[:])

    _split_multiwaits(nc)
    return nc


# ------------------------------------------------------------------ entry
def kernel(**inputs):
    import ml_dtypes

    import time as _time
    _t0 = _time.time()
    x = np.asarray(inputs["x"], np.float32)
    S = _prep(np.asarray(inputs["edge_index"]), np.asarray(inputs["batch"]))
    cores = _core_inputs(S, x)
    T_win = S["T_win"]
    if os.environ.get("K_TIME") == "1":
        print(f"[k] host prep: {_time.time()-_t0:.2f}s")

    w1p = np.zeros((4, 32), np.float32)
    w1p[:3] = np.asarray(inputs["W1"], np.float32)
    common = dict(w1=w1p.astype(ml_dtypes.bfloat16),
                  w2=np.asarray(inputs["W2"], np.float32).astype(ml_dtypes.bfloat16),
                  w3=np.asarray(inputs["W3"], np.float32).astype(ml_dtypes.bfloat16),
                  linw=np.asarray(inputs["lin_w"], np.float32),
                  linb_pb=np.broadcast_to(
                      np.asarray(inputs["lin_b"], np.float32), (32, 3)).copy())
    for i in range(3):
        ga = np.asarray(inputs[f"gn{i+1}_a"], np.float32)
        gw = np.asarray(inputs[f"gn{i+1}_w"], np.float32)
        gb = np.asarray(inputs[f"gn{i+1}_b"], np.float32)
        bc = np.asarray(inputs[f"b{i+1}"], np.float32)
        common[f"gnc{i+1}"] = np.stack(
            [-ga, (1.0 - ga) * bc, gw, gb], axis=1).astype(np.float32)
    ofix = (np.arange(128)[:, None] // 4 == np.arange(32)[None, :])
    common["ofix_f"] = ofix.astype(np.float32)
    common["ofix_h"] = ofix.astype(ml_dtypes.bfloat16)

    global _NC_CACHE
    try:
        _NC_CACHE
    except NameError:
        _NC_CACHE = {}
    key = (T_win, tuple(S["tile_block"].tolist()))
    if key not in _NC_CACHE:
        _NC_CACHE[key] = _build_nc(T_win, S["tile_block"])
    nc = _NC_CACHE[key]

    in_maps = []
    for c in range(NCORES):
        m = dict(common)
        m.update(cores[c])
        in_maps.append(m)

    from concourse.bass_utils import run_bass_kernel_spmd
    import time as _time
    trace = os.environ.get("K_TRACE") == "1"
    _t0 = _time.time()
    try:
        res = run_bass_kernel_spmd(nc, in_maps, list(range(NCORES)), trace=trace)
    except ModuleNotFoundError:
        res = run_bass_kernel_spmd(nc, in_maps, list(range(NCORES)))
    if os.environ.get("K_TIME") == "1":
        print(f"[k] run_bass_kernel_spmd: {_time.time()-_t0:.2f}s")
    if trace:
        try:
            print("exec_time_ns:", res.exec_time_ns)
        except Exception:
            pass
    out = np.concatenate([np.asarray(res.results[c]["out"])
                          for c in range(NCORES)], axis=0)
    return out.astype(np.float32)


if __name__ == "__main__":
    import sys
    sys.path.insert(0, "/root/problem")
    os.environ["JAX_PLATFORMS"] = "cpu"
    import jax
    jax.config.update("jax_platforms", "cpu")
    import reference
    inputs = {k: np.asarray(v) for k, v in reference.setup_inputs().items()}
    exp = np.asarray(reference.reference(**inputs))
    act = kernel(**inputs)
    err = np.abs(act - exp).max() / np.abs(exp).max()
    print(f"Relative error: {err:.3e}")



# revision 2
# speedup vs baseline: 39.5671x; 39.5671x over previous
"""Trainium2 Bass kernel for GCNGraphClassifier (3x GCNConv+GraphNorm+ReLU,
global_mean_pool, linear head). 8-core SPMD.

Self-contained: host preprocessing (graph partitioning, banded edge
schedule, norm factorization) + Bass/Tile device program.

Sharding: graphs block-partitioned to cores (32/core); each graph padded
to a 512-column window; node columns degree-packed into 16 col-blocks with
a globally uniform tile allotment so the SPMD instruction stream is
identical across cores. Message aggregation = PE matmuls against a fixed
0/1 pattern; gathers via indirect DMA from a replicated bf16 node table
(AllGather between layers). GCN norm factorized into per-node pre/post
scales (dinv), so edge messages need no per-edge multiply.
"""

import os
import numpy as np

N = 100000
ERAW = 1600000
G = 256
NCORES = 8
GPC = 32
WCOLS = 512
CPC = GPC * WCOLS  # 16384
EPS = 1e-5
FINS = (4, 32, 64)
FOUTS = (32, 64, 128)


# ------------------------------------------------------------------ host prep
def _prep(edge_index, batch):
    src = np.asarray(edge_index[0], dtype=np.int64)
    dst = np.asarray(edge_index[1], dtype=np.int64)
    batch = np.asarray(batch, dtype=np.int64)
    loops = np.arange(N, dtype=np.int64)
    src = np.concatenate([src, loops])
    dst = np.concatenate([dst, loops])
    deg = np.bincount(dst, minlength=N)
    dinv = (1.0 / np.sqrt(np.maximum(deg, 1.0))).astype(np.float64)
    dinv[deg == 0] = 0.0

    order = np.argsort(dst, kind="stable")
    src_s = src[order]
    dst_s = dst[order]
    starts = np.zeros(N + 1, dtype=np.int64)
    np.cumsum(np.bincount(dst, minlength=N), out=starts[1:])

    gcnt = np.bincount(batch, minlength=G).astype(np.int64)
    gstart = np.zeros(G + 1, dtype=np.int64)
    np.cumsum(gcnt, out=gstart[1:])
    assert gcnt.max() <= WCOLS

    need = -(-deg // 4)  # ceil(deg/4), tiles per node

    # column packing: per graph, nodes sorted by degree desc -> col ranks
    col_of = np.full(N, -1, dtype=np.int64)
    node_at = np.full((G, WCOLS), -1, dtype=np.int64)
    blockneed = np.zeros((G, 16), dtype=np.int64)
    for g in range(G):
        nodes = np.arange(gstart[g], gstart[g + 1])
        nodes = nodes[np.argsort(-deg[nodes], kind="stable")]
        cols = np.arange(len(nodes))
        col_of[nodes] = cols
        node_at[g, : len(nodes)] = nodes
        np.maximum.at(blockneed[g], cols // 32, need[nodes])
    A = blockneed.max(axis=0)
    T_win = int(A.sum())
    tstart = np.zeros(17, dtype=np.int64)
    np.cumsum(A, out=tstart[1:])
    tile_block = np.repeat(np.arange(16), A)
    T_total = T_win * GPC

    core_of = batch // GPC
    row_of = core_of * CPC + (batch % GPC) * WCOLS + col_of
    PAD_ROW = WCOLS - 1  # core0 win0 col511: guaranteed pad (dinv=0 -> zeros)
    assert node_at[0, WCOLS - 1] == -1

    # vectorized per-edge slot assignment
    e_node = dst_s
    e_rank = np.arange(len(dst_s)) - starts[e_node]
    e_core = core_of[e_node]
    e_w = (batch[e_node] % GPC)
    e_col = col_of[e_node]
    e_b = e_col // 32
    e_ci = e_col % 32
    e_k = e_rank // 4
    e_j = e_rank % 4
    e_t = e_w * T_win + tstart[e_b] + e_k
    e_r = 4 * e_ci + e_j
    offs = np.full((NCORES, 128, T_total), PAD_ROW, dtype=np.int32)
    offs[e_core, e_r, e_t] = row_of[src_s].astype(np.int32)
    return dict(dinv=dinv, gcnt=gcnt, node_at=node_at, A=A, T_win=T_win,
                T_total=T_total, tile_block=tile_block, offs=offs,
                e=(e_core, e_r, e_t, src_s), batch=batch)


def _core_inputs(S, x):
    import ml_dtypes
    dinv = S["dinv"]
    T_total = S["T_total"]
    x4 = np.zeros((N, 4), np.float32)
    x4[:, :3] = x
    gx = (dinv[:, None] * x4).astype(np.float32)
    e_core, e_r, e_t, src_s = S["e"]

    xe_all = np.zeros((NCORES, 128, T_total, 4), np.float32)
    xe_all[e_core, e_r, e_t] = gx[src_s]

    cores = []
    for c in range(NCORES):
        dcols = np.zeros(CPC, np.float64)
        for w in range(GPC):
            nodes = S["node_at"][c * GPC + w]
            msk = nodes >= 0
            dcols[w * WCOLS:(w + 1) * WCOLS][msk] = dinv[nodes[msk]]
        gcnt_c = S["gcnt"][c * GPC:(c + 1) * GPC].astype(np.float32)
        invcnt = (1.0 / np.maximum(gcnt_c, 1.0)).astype(np.float32)
        npad = (WCOLS - gcnt_c).astype(np.float32)
        cores.append(dict(
            xe=np.ascontiguousarray(xe_all[c].reshape(128, T_total * 4)).astype(ml_dtypes.bfloat16),
            offs=np.ascontiguousarray(S["offs"][c]),
            dinvb=np.broadcast_to(dcols.astype(np.float32), (64, CPC)).copy(),
            dinv_nm=np.ascontiguousarray(
                dcols.astype(np.float32).reshape(CPC // 128, 128).T),
            invcnt_pb=np.broadcast_to(invcnt, (128, GPC)).copy(),
            npad_pb=np.broadcast_to(npad, (128, GPC)).copy(),
        ))
    return cores


def _split_multiwaits(nc):
    """This walrus build accepts at most one sync-wait per instruction
    struct; split extras onto same-engine NoOps inserted just before."""
    import concourse.mybir as mybir
    k = 0
    for bass_bb in nc.bb_map.values():
        bb = bass_bb.bb if hasattr(bass_bb, "bb") else bass_bb
        out = []
        changed = False
        for ins in bb.instructions:
            si = getattr(ins, "sync_info", None)
            if si is not None and si.on_wait is not None and len(si.on_wait) > 1:
                waits = list(si.on_wait)
                for wsub in waits[:-1]:
                    k += 1
                    nop = mybir.InstNoOp(name=f"WNOP-{k}", engine=ins.engine,
                                         ins=[], outs=[])
                    nop.sync_info = mybir.SyncInfo(on_wait=[wsub], on_update=[])
                    out.append(nop)
                ins.sync_info = mybir.SyncInfo(
                    on_wait=[waits[-1]], on_update=list(si.on_update))
                changed = True
            out.append(ins)
        if changed:
            bb.instructions = out


# --------------------------------------------------------------- bass program
def _build_nc(T_win, tile_block):
    import concourse.bass as bass
    import concourse.mybir as mybir
    from concourse.tile import TileContext
    from concourse.masks import make_identity

    f32 = mybir.dt.float32
    bf16 = mybir.dt.bfloat16
    i32 = mybir.dt.int32
    AX = mybir.AxisListType.X
    OP = mybir.AluOpType
    AF = mybir.ActivationFunctionType
    T_total = T_win * GPC

    nc = bass.Bass()
    xe_d = nc.declare_dram_parameter("xe", [128, T_total * 4], bf16, isOutput=False)
    offs_d = nc.declare_dram_parameter("offs", [128, T_total], i32, isOutput=False)
    dinvb_d = nc.declare_dram_parameter("dinvb", [64, CPC], f32, isOutput=False)
    dinvnm_d = nc.declare_dram_parameter("dinv_nm", [128, CPC // 128], f32, isOutput=False)
    invcnt_d = nc.declare_dram_parameter("invcnt_pb", [128, GPC], f32, isOutput=False)
    npad_d = nc.declare_dram_parameter("npad_pb", [128, GPC], f32, isOutput=False)
    w1_d = nc.declare_dram_parameter("w1", [4, 32], bf16, isOutput=False)
    w2_d = nc.declare_dram_parameter("w2", [32, 64], bf16, isOutput=False)
    w3_d = nc.declare_dram_parameter("w3", [64, 128], bf16, isOutput=False)
    linw_d = nc.declare_dram_parameter("linw", [128, 3], f32, isOutput=False)
    linb_d = nc.declare_dram_parameter("linb_pb", [32, 3], f32, isOutput=False)
    # gn consts per layer: cols = [neg_alpha, c=(1-a)b, w_gn, b_gn]
    gnc_d = [nc.declare_dram_parameter(f"gnc{i+1}", [FOUTS[i], 4], f32,
                                       isOutput=False) for i in range(3)]
    ofixf_d = nc.declare_dram_parameter("ofix_f", [128, 32], f32, isOutput=False)
    ofixh_d = nc.declare_dram_parameter("ofix_h", [128, 32], bf16, isOutput=False)
    out_d = nc.declare_dram_parameter("out", [32, 3], f32, isOutput=True)

    tbl_sh = [nc.dram_tensor(f"tbl{i}_sh", [CPC, FINS[i]], bf16)
              for i in (1, 2)]
    tbl_full = [nc.dram_tensor(f"tbl{i}_full", [NCORES * CPC, FINS[i]], bf16,
                               addr_space="Shared") for i in (1, 2)]

    RG = [list(range(NCORES))]

    with TileContext(nc) as tc:
        with (
            tc.tile_pool(name="const", bufs=1) as cpool,
            tc.tile_pool(name="big", bufs=1) as bigpool,
            tc.tile_pool(name="sb", bufs=2) as sb,
            tc.tile_pool(name="stat", bufs=12) as st,
            tc.tile_pool(name="ps_agg", bufs=2, space="PSUM") as ps_agg,
            tc.tile_pool(name="ps_dn", bufs=2, space="PSUM") as ps_dn,
            tc.tile_pool(name="ps_tp", bufs=2, space="PSUM") as ps_tp,
            tc.tile_pool(name="gpool", bufs=2) as gpool,
        ):
            # ---- constants to SBUF
            def load(pool, dram, shape, dtype, tag):
                t = pool.tile(shape, dtype, tag=tag)
                nc.sync.dma_start(out=t[:], in_=dram[:])
                return t

            ofix_f = load(cpool, ofixf_d, [128, 32], f32, "ofix_f")
            ofix_h = load(cpool, ofixh_d, [128, 32], bf16, "ofix_h")
            offs_sb = load(cpool, offs_d, [128, T_total], i32, "offs")
            dinvb = load(cpool, dinvb_d, [64, CPC], f32, "dinvb")
            dinvnm = load(cpool, dinvnm_d, [128, CPC // 128], f32, "dinvnm")
            invcnt = load(cpool, invcnt_d, [128, GPC], f32, "invcnt")
            npad = load(cpool, npad_d, [128, GPC], f32, "npad")
            w1 = load(cpool, w1_d, [4, 32], bf16, "w1")
            w2 = load(cpool, w2_d, [32, 64], bf16, "w2")
            w3 = load(cpool, w3_d, [64, 128], bf16, "w3")
            linw = load(cpool, linw_d, [128, 3], f32, "linw")
            linb = load(cpool, linb_d, [32, 3], f32, "linb")
            gnc = [load(cpool, gnc_d[i], [FOUTS[i], 4], f32, f"gnc{i}")
                   for i in range(3)]
            Ws = [w1, w2, w3]

            ident = cpool.tile([128, 128], f32)
            make_identity(nc, ident[:])
            zl = cpool.tile([128, 64], f32)
            nc.vector.memset(zl[:], 0.0)
            zn = cpool.tile([128, 512], f32)
            nc.vector.memset(zn[:], 0.0)
            zlh = cpool.tile([128, 64], bf16)
            nc.vector.memset(zlh[:], 0.0)
            znh = cpool.tile([128, 512], bf16)
            nc.vector.memset(znh[:], 0.0)
            epsc = cpool.tile([128, 1], f32)
            nc.vector.memset(epsc[:], EPS)

            # per-engine const warmups: absorb each const tile's DMA wait
            # onto its consuming engine once (walrus: <=1 sync wait per inst)
            scrd = cpool.tile([128, 8], f32, tag="scrd")
            for ap in (dinvb[:64, :1], dinvnm[:, :1], invcnt[:, :1],
                       npad[:, :1], linb[:32, :1], gnc[0][:, :1],
                       gnc[1][:, :1], gnc[2][:, :1], epsc[:, :1]):
                nc.vector.tensor_copy(out=scrd[:ap.shape[0], :1], in_=ap)
            scra = cpool.tile([128, 8], f32, tag="scra")
            nc.scalar.activation(out=scra[:, :1], in_=epsc[:, :], func=AF.Copy)
            for i in range(3):
                nc.scalar.activation(out=scra[:FOUTS[i], 1:2], in_=gnc[i][:, :1],
                                     func=AF.Copy)
            scrg = cpool.tile([128, 8], i32, tag="scrg")
            nc.gpsimd.tensor_copy(out=scrg[:, :1], in_=offs_sb[:, :1])

            # wait-absorbers: pull const-DMA/identity deps onto PE early so
            # no later matmul carries more than one sync wait (LW struct limit)
            tr0 = ps_tp.tile([128, 512], f32, tag="tp")
            nc.tensor.transpose(out=tr0[:, :128], in_=ident[:, :], identity=ident[:, :])
            nc.tensor.matmul(out=tr0[:3, :3], lhsT=linw[:, :], rhs=linw[:, :],
                             start=True, stop=True)
            nc.tensor.ldweights(weights=w1[:, :])
            nc.tensor.ldweights(weights=w2[:, :])
            nc.tensor.ldweights(weights=w3[:, :])
            nc.tensor.ldweights(weights=ofix_h[:, :])

            p_sb = bigpool.tile([64, CPC], bf16)      # aggregation out (post-scaled)
            pooled = bigpool.tile([128, GPC], f32)   # layer3 pooled sums

            # ---------------- helpers ----------------
            GRP = 4  # windows per gather group (SBUF: GRP*T_win*64*2B/part)

            def aggregate(lay, Fin):
                """phase 1: all gathers for the layer into one big buffer;
                phase 2: O-pattern matmuls per window; p_sb <- psum*dinv."""
                fp32 = lay == 0
                of = ofix_h
                zlx, znx = (zlh, znh)
                for g0 in range(0, GPC, GRP):
                    gbig = gpool.tile([128, GRP * T_win * 64], bf16, tag="gbig")
                    tbase = g0 * T_win
                    ngrp = GRP * T_win
                    if fp32:
                        nc.sync.dma_start(
                            out=gbig[:, :ngrp * 4],
                            in_=xe_d[:, tbase * 4:(tbase + ngrp) * 4])
                    elif os.environ.get("K_NOGATHER") == "1":
                        nc.sync.dma_start(
                            out=gbig[:, :ngrp * Fin],
                            in_=tbl_full[lay - 1][:ngrp, :]
                            .rearrange("t f -> () (t f)").to_broadcast(
                                [128, ngrp * Fin]))
                    else:
                        for t in range(ngrp):
                            nc.gpsimd.indirect_dma_start(
                                out=gbig[:, t * Fin:(t + 1) * Fin],
                                out_offset=None,
                                in_=tbl_full[lay - 1][:, :],
                                in_offset=bass.IndirectOffsetOnAxis(
                                    ap=offs_sb[:, tbase + t:tbase + t + 1],
                                    axis=0),
                            )
                    nc.tensor.ldweights(weights=gbig[:, :Fin])
                    for w in range(g0, g0 + GRP):
                        ps = ps_agg.tile([64, 512], f32, tag="agg")
                        nc.tensor.matmul(out=ps[:Fin, :], lhsT=zlx[:, :Fin],
                                         rhs=znx[:], start=True, stop=False)
                        for t in range(T_win):
                            b = tile_block[t]
                            tl = (w - g0) * T_win + t
                            nc.tensor.matmul(
                                out=ps[:Fin, 32 * b:32 * b + 32],
                                lhsT=gbig[:, tl * Fin:(tl + 1) * Fin],
                                rhs=of[:], start=False, stop=(t == T_win - 1))
                        nc.vector.tensor_tensor(
                            out=p_sb[:Fin, w * WCOLS:(w + 1) * WCOLS],
                            in0=ps[:Fin, :],
                            in1=dinvb[:Fin, w * WCOLS:(w + 1) * WCOLS],
                            op=OP.mult)

            def dense_gn(lay, Fin, Fo, last):
                W, gc = Ws[lay], gnc[lay]
                for w in range(GPC):
                    wsl = slice(w * WCOLS, (w + 1) * WCOLS)
                    px = ps_dn.tile([128, 512], f32, tag="dense")
                    nc.tensor.matmul(out=px[:Fo, :], lhsT=W[:, :],
                                     rhs=p_sb[:Fin, wsl], start=True, stop=True)
                    # stats via ACT accumulate
                    scr = sb.tile([128, 512], f32, tag="scr")
                    ssum = st.tile([128, 1], f32, tag="ssum")
                    ssq = st.tile([128, 1], f32, tag="ssq")
                    nc.scalar.activation(out=scr[:Fo, :], in_=px[:Fo, :],
                                         func=AF.Copy, accum_out=ssum[:Fo, :])
                    nc.scalar.activation(out=scr[:Fo, :], in_=px[:Fo, :],
                                         func=AF.Square, accum_out=ssq[:Fo, :])
                    # scalar math [Fo,1]
                    m = st.tile([128, 1], f32, tag="m")
                    qm = st.tile([128, 1], f32, tag="qm")
                    d = st.tile([128, 1], f32, tag="d")
                    t1 = st.tile([128, 1], f32, tag="t1")
                    var = st.tile([128, 1], f32, tag="var")
                    istd = st.tile([128, 1], f32, tag="istd")
                    s1 = st.tile([128, 1], f32, tag="s1")
                    s2 = st.tile([128, 1], f32, tag="s2")
                    nc.vector.tensor_scalar(out=m[:Fo], in0=ssum[:Fo],
                                            scalar1=invcnt[:Fo, w:w + 1],
                                            scalar2=None, op0=OP.mult)
                    nc.vector.tensor_scalar(out=qm[:Fo], in0=ssq[:Fo],
                                            scalar1=invcnt[:Fo, w:w + 1],
                                            scalar2=None, op0=OP.mult)
                    # d = m*neg_alpha + c
                    nc.vector.tensor_scalar(out=d[:Fo], in0=m[:Fo],
                                            scalar1=gc[:, 0:1], scalar2=gc[:, 1:2],
                                            op0=OP.mult, op1=OP.add)
                    # var = qm + d*(2m + d)
                    nc.vector.tensor_scalar(out=t1[:Fo], in0=m[:Fo], scalar1=2.0,
                                            scalar2=d[:Fo], op0=OP.mult, op1=OP.add)
                    nc.vector.tensor_tensor(out=t1[:Fo], in0=t1[:Fo], in1=d[:Fo],
                                            op=OP.mult)
                    nc.vector.tensor_tensor(out=var[:Fo], in0=qm[:Fo], in1=t1[:Fo],
                                            op=OP.add)
                    nc.scalar.activation(out=istd[:Fo], in_=var[:Fo], func=AF.Sqrt,
                                         bias=epsc[:Fo, :])
                    nc.vector.reciprocal(out=istd[:Fo], in_=istd[:Fo])
                    nc.vector.tensor_scalar(out=s1[:Fo], in0=istd[:Fo],
                                            scalar1=gc[:, 2:3], scalar2=None,
                                            op0=OP.mult)
                    nc.vector.tensor_scalar(out=s2[:Fo], in0=d[:Fo],
                                            scalar1=s1[:Fo], scalar2=gc[:, 3:4],
                                            op0=OP.mult, op1=OP.add)
                    # h_pre = px*s1 + s2 ; relu
                    hw = sb.tile([128, 512], f32, tag="hw")
                    nc.vector.tensor_scalar(out=hw[:Fo, :], in0=px[:Fo, :],
                                            scalar1=s1[:Fo], scalar2=s2[:Fo],
                                            op0=OP.mult, op1=OP.add)
                    if last:
                        hr = sb.tile([128, 512], f32, tag="hr")
                        psum_col = st.tile([128, 1], f32, tag="pool1")
                        nc.scalar.activation(out=hr[:Fo, :], in_=hw[:Fo, :],
                                             func=AF.Relu,
                                             accum_out=psum_col[:Fo, :])
                        # pooled -= npad * relu(s2); then *invcnt
                        rs2 = st.tile([128, 1], f32, tag="rs2")
                        nc.vector.tensor_scalar(out=rs2[:Fo], in0=s2[:Fo],
                                                scalar1=0.0, scalar2=npad[:Fo, w:w + 1],
                                                op0=OP.max, op1=OP.mult)
                        nc.vector.tensor_tensor(out=psum_col[:Fo], in0=psum_col[:Fo],
                                                in1=rs2[:Fo], op=OP.subtract)
                        nc.vector.tensor_scalar(out=pooled[:Fo, w:w + 1],
                                                in0=psum_col[:Fo],
                                                scalar1=invcnt[:Fo, w:w + 1],
                                                scalar2=None, op0=OP.mult)
                    else:
                        hr = sb.tile([128, 512], f32, tag="hr")
                        nc.scalar.activation(out=hr[:Fo, :], in_=hw[:Fo, :],
                                             func=AF.Relu)
                        # transpose 4x [Fo,128] -> [128,Fo], prescale, -> table
                        tp = ps_tp.tile([128, 512], f32, tag="tp")
                        tb = sb.tile([128, 4 * Fo], bf16, tag="tb")
                        for ccc in range(4):
                            nc.tensor.transpose(
                                out=tp[:, ccc * Fo:(ccc + 1) * Fo],
                                in_=hr[:Fo, 128 * ccc:128 * (ccc + 1)],
                                identity=ident[:Fo, :Fo])
                            nc.vector.tensor_scalar(
                                out=tb[:, ccc * Fo:(ccc + 1) * Fo],
                                in0=tp[:, ccc * Fo:(ccc + 1) * Fo],
                                scalar1=dinvnm[:, 4 * w + ccc:4 * w + ccc + 1],
                                scalar2=None, op0=OP.mult)
                        nc.sync.dma_start(
                            out=tbl_sh[lay][w * WCOLS:(w + 1) * WCOLS, :]
                            .rearrange("(c p) f -> p c f", p=128),
                            in_=tb[:].rearrange("p (c f) -> p c f", f=Fo))

            NOCC = os.environ.get("K_NOCC") == "1"
            NOGATHER = os.environ.get("K_NOGATHER") == "1"
            ONELAYER = os.environ.get("K_ONELAYER") == "1"

            def distribute(i):
                if NOCC:
                    # local-only stand-in (wrong answer, bisect aid)
                    nc.sync.dma_start(out=tbl_full[i][:CPC, :],
                                      in_=tbl_sh[i][:, :])
                else:
                    nc.gpsimd.collective_compute(
                        "AllGather", OP.bypass, ins=[tbl_sh[i][:, :]],
                        outs=[tbl_full[i][:, :]], replica_groups=RG)

            # ---------------- layer 1 ----------------
            aggregate(0, 4)
            dense_gn(0, 4, 32, last=False)
            if not ONELAYER:
                distribute(0)
                # ---------------- layer 2 ----------------
                aggregate(1, 32)
                dense_gn(1, 32, 64, last=False)
                distribute(1)
                # ---------------- layer 3 ----------------
                aggregate(2, 64)
                dense_gn(2, 64, 128, last=True)

            # ---------------- head ----------------
            hps = ps_tp.tile([32, 3], f32, tag="head")
            nc.tensor.matmul(out=hps[:, :], lhsT=pooled[:, :], rhs=linw[:, :],
                             start=True, stop=True)
            ob = sb.tile([32, 3], f32, tag="ob")
            nc.vector.tensor_tensor(out=ob[:], in0=hps[:], in1=linb[:], op=OP.add)
            nc.sync.dma_start(out=out_d[:, :], in_=ob[:])

    _split_multiwaits(nc)
    return nc


# ------------------------------------------------------------------ entry
def kernel(**inputs):
    import ml_dtypes

    import time as _time
    _t0 = _time.time()
    x = np.asarray(inputs["x"], np.float32)
    S = _prep(np.asarray(inputs["edge_index"]), np.asarray(inputs["batch"]))
    cores = _core_inputs(S, x)
    T_win = S["T_win"]
    if os.environ.get("K_TIME") == "1":
        print(f"[k] host prep: {_time.time()-_t0:.2f}s")

    w1p = np.zeros((4, 32), np.float32)
    w1p[:3] = np.asarray(inputs["W1"], np.float32)
    common = dict(w1=w1p.astype(ml_dtypes.bfloat16),
                  w2=np.asarray(inputs["W2"], np.float32).astype(ml_dtypes.bfloat16),
                  w3=np.asarray(inputs["W3"], np.float32).astype(ml_dtypes.bfloat16),
                  linw=np.asarray(inputs["lin_w"], np.float32),
                  linb_pb=np.broadcast_to(
                      np.asarray(inputs["lin_b"], np.float32), (32, 3)).copy())
    for i in range(3):
        ga = np.asarray(inputs[f"gn{i+1}_a"], np.float32)
        gw = np.asarray(inputs[f"gn{i+1}_w"], np.float32)
        gb = np.asarray(inputs[f"gn{i+1}_b"], np.float32)
        bc = np.asarray(inputs[f"b{i+1}"], np.float32)
        common[f"gnc{i+1}"] = np.stack(
            [-ga, (1.0 - ga) * bc, gw, gb], axis=1).astype(np.float32)
    ofix = (np.arange(128)[:, None] // 4 == np.arange(32)[None, :])
    common["ofix_f"] = ofix.astype(np.float32)
    common["ofix_h"] = ofix.astype(ml_dtypes.bfloat16)

    global _NC_CACHE
    try:
        _NC_CACHE
    except NameError:
        _NC_CACHE = {}
    key = (T_win, tuple(S["tile_block"].tolist()))
    if key not in _NC_CACHE:
        _NC_CACHE[key] = _build_nc(T_win, S["tile_block"])
    nc = _NC_CACHE[key]

    in_maps = []
    for c in range(NCORES):
        m = dict(common)
        m.update(cores[c])
        in_maps.append(m)

    from concourse.bass_utils import run_bass_kernel_spmd
    import time as _time
    trace = os.environ.get("K_TRACE") == "1"
    _t0 = _time.time()
    kw = {}
    if os.environ.get("K_TMPDIR"):
        kw["tmpdir"] = os.environ["K_TMPDIR"]
    try:
        res = run_bass_kernel_spmd(nc, in_maps, list(range(NCORES)), trace=trace, **kw)
    except ModuleNotFoundError:
        res = run_bass_kernel_spmd(nc, in_maps, list(range(NCORES)))
    if os.environ.get("K_TIME") == "1":
        print(f"[k] run_bass_kernel_spmd: {_time.time()-_t0:.2f}s")
    if trace:
        try:
            print("exec_time_ns:", res.exec_time_ns)
        except Exception:
            pass
    out = np.concatenate([np.asarray(res.results[c]["out"])
                          for c in range(NCORES)], axis=0)
    return out.astype(np.float32)


if __name__ == "__main__":
    import sys
    sys.path.insert(0, "/root/problem")
    os.environ["JAX_PLATFORMS"] = "cpu"
    import jax
    jax.config.update("jax_platforms", "cpu")
    import reference
    inputs = {k: np.asarray(v) for k, v in reference.setup_inputs().items()}
    exp = np.asarray(reference.reference(**inputs))
    act = kernel(**inputs)
    err = np.abs(act - exp).max() / np.abs(exp).max()
    print(f"Relative error: {err:.3e}")

